# revision 19
# baseline (speedup 1.0000x reference)
"""Trainium2 Bass kernel for CentersDistance (vq_codebook).

logits[c, q] = -||centers[c] - inputs[q]||^2  for inputs [4096,128], centers [256,128].

Sharding (per spec hint): shard inputs along Q across 8 cores (512
queries/core), replicate centers; each core computes its [C, 512] slab
independently, no collectives.

kernel() prepares per-core layouts on the host (sharding-time layout/
precision prep; the O(C*Q*D) distance FLOPs stay on device). The fp32
operands are split into bf16 hi/lo planes (x ~= hi + lo to ~2^-18 rel), so
the device reconstructs the fp32-precision dot with three full-rate bf16
matmuls per tile (hi*hi + hi*lo + lo*hi in fp32 PSUM; the dropped lo*lo
term is inside the fp32 envelope). Each HWDGE ring gets ONE packed bf16
input tensor (same byte count as fp32):
  sync ring  : xq [128, 1032] = [2xT_hi | 2xT_lo | -qnorm (fp32 bitcast)]
  scalar ring: cn [128, 1024] = [cT_hi | cT_lo | -cnorm replicated (fp32)]

Per-core device graph (logits = (2x).c - ||x||^2 - ||c||^2, computed as
outT[q, c] then transposed on gather), per q-chunk n:
  - three accumulating bf16 matmuls into the chunk's own PSUM bank
  - DVE bias_n[p,c] = -qnorm[p,n] + -cnorm[c]; DVE add psum_n + bias_n
  - output DMA on alternating rings, overlapping later chunks' compute
"""

import ml_dtypes
import numpy as np
from contextlib import ExitStack

import concourse.bass as bass
import concourse.bacc as bacc
import concourse.tile as tile
from concourse import mybir
from concourse.bass_utils import run_bass_kernel_spmd

Q, C, D = 4096, 256, 128
NCORES = 8
QL = Q // NCORES      # 512 queries per core
NQ = QL // 128        # 4 query chunks per core
F32 = mybir.dt.float32
BF16 = mybir.dt.bfloat16

NRM = 2 * NQ              # -qnorm: NQ fp32 values = 2*NQ bf16 cols
XCOLS = 2 * QL + NRM      # hi | lo | norms
CCOLS = 2 * C + 2 * C     # c_hi | c_lo | fp32 cnorm replica

_NC = None
LAST_RESULTS = None


def _build_nc():
    nc = bacc.Bacc("TRN2", target_bir_lowering=False)
    xq = nc.declare_dram_parameter("xq", [D, XCOLS], BF16, isOutput=False)
    cn = nc.declare_dram_parameter("cn", [D, CCOLS], BF16, isOutput=False)
    out = nc.declare_dram_parameter("out", [QL, C], F32, isOutput=True)

    with ExitStack() as ctx:
        tc = ctx.enter_context(tile.TileContext(nc))
        const = ctx.enter_context(tc.tile_pool(name="const", bufs=1))
        outp = ctx.enter_context(tc.tile_pool(name="outp", bufs=4))
        bp = ctx.enter_context(tc.tile_pool(name="bp", bufs=4))
        pm = ctx.enter_context(
            tc.tile_pool(name="pm", bufs=4, space=bass.MemorySpace.PSUM)
        )

        cn_sb = const.tile([D, CCOLS], BF16)
        nc.scalar.dma_start(cn_sb[:], cn[:, :])
        xq_sb = const.tile([D, XCOLS], BF16)
        nc.sync.dma_start(xq_sb[:], xq[:, :])

        c_hi = cn_sb[:, 0:C]
        c_lo = cn_sb[:, C : 2 * C]
        ncr = cn_sb[:, 2 * C : CCOLS].bitcast(F32)                  # [128, C]
        nqn = xq_sb[:, 2 * QL : XCOLS].bitcast(F32)                 # [128, NQ]

        # bias[p,n,c] = -qnorm[p,n] + -cnorm[c]
        bias = bp.tile([128, NQ, C], F32)
        nc.vector.tensor_add(
            bias[:],
            nqn.rearrange("p (n o) -> p n o", o=1).broadcast_to([128, NQ, C]),
            ncr.rearrange("p (o c) -> p o c", o=1).broadcast_to([128, NQ, C]),
        )

        # chunk-pipelined mains + epilogue; each chunk owns a PSUM bank
        for n in range(NQ):
            x_hi = xq_sb[:, bass.ts(n, 128)]
            x_lo = xq_sb[:, QL + n * 128 : QL + (n + 1) * 128]
            ps = pm.tile([128, C], F32, tag="ps")
            nc.tensor.matmul(ps[:], x_hi, c_hi, start=True, stop=False)
            nc.tensor.matmul(ps[:], x_hi, c_lo, start=False, stop=False)
            nc.tensor.matmul(ps[:], x_lo, c_hi, start=False, stop=True)
            o = outp.tile([128, C], F32, tag="o")
            nc.vector.tensor_add(o[:], ps[:], bias[:, n, :])
            eng = nc.sync if n % 2 == 0 else nc.scalar
            eng.dma_start(out[bass.ts(n, 128), :], o[:])

    nc.compile()  # Bacc register allocation; walrus rejects unallocated regs
    return nc


def get_nc():
    global _NC
    if _NC is None:
        _NC = _build_nc()
    return _NC


def _split_bf16(a):
    """a (fp32) -> (hi, lo) bf16 with hi + lo ~= a to ~2^-18 rel."""
    hi = a.astype(ml_dtypes.bfloat16)
    lo = (a - hi.astype(np.float32)).astype(ml_dtypes.bfloat16)
    return hi, lo


def _pack_inputs(inputs, centers):
    cT = np.ascontiguousarray(centers.T)                              # [D, C]
    c_hi, c_lo = _split_bf16(cT)
    ncn = -(centers.astype(np.float64) ** 2).sum(1).astype(np.float32)
    ncr = np.ascontiguousarray(
        np.broadcast_to(ncn[None, :], (128, C)).astype(np.float32)
    )
    cn_pack = np.ascontiguousarray(
        np.concatenate([c_hi, c_lo, ncr.view(ml_dtypes.bfloat16)], axis=1)
    )
    maps = []
    for i in range(NCORES):
        xs = inputs[i * QL : (i + 1) * QL]
        x_hi, x_lo = _split_bf16(np.ascontiguousarray(2.0 * xs.T))  # [D, QL]
        nq = -(xs.astype(np.float64) ** 2).sum(1).astype(np.float32)
        nq_cols = np.ascontiguousarray(nq.reshape(NQ, 128).T)       # [128, NQ]
        xq_pack = np.ascontiguousarray(np.concatenate(
            [x_hi, x_lo, nq_cols.view(ml_dtypes.bfloat16)], axis=1
        ))
        maps.append({"xq": xq_pack, "cn": cn_pack})
    return maps


def kernel(inputs: np.ndarray, centers: np.ndarray, trace: bool = False):
    global LAST_RESULTS
    inputs = np.asarray(inputs, dtype=np.float32)
    centers = np.asarray(centers, dtype=np.float32)
    assert inputs.shape == (Q, D) and centers.shape == (C, D)

    nc_ = get_nc()
    in_maps = _pack_inputs(inputs, centers)
    res = run_bass_kernel_spmd(nc_, in_maps, list(range(NCORES)), trace=trace)
    LAST_RESULTS = res
    full = np.empty((C, Q), dtype=np.float32)
    for i in range(NCORES):
        full[:, i * QL : (i + 1) * QL] = res.results[i]["out"].T
    return full


# revision 20
# speedup vs baseline: 1.0430x; 1.0430x over previous
"""Trainium2 Bass kernel for CentersDistance (vq_codebook).

logits[c, q] = -||centers[c] - inputs[q]||^2  for inputs [4096,128], centers [256,128].

Sharding (per spec hint): shard inputs along Q across 8 cores (512
queries/core), replicate centers; each core computes its [C, 512] slab
independently, no collectives.

kernel() prepares per-core layouts on the host (sharding-time layout/
precision prep; the O(C*Q*D) distance FLOPs stay on device). The fp32
operands are split into bf16 hi/lo planes (x ~= hi + lo to ~2^-18 rel), so
the device reconstructs the fp32-precision dot with three full-rate bf16
matmuls per tile (hi*hi + hi*lo + lo*hi in fp32 PSUM; the dropped lo*lo
term is inside the fp32 envelope). Each HWDGE ring gets ONE packed bf16
input tensor (same byte count as fp32):
  sync ring  : xq [128, 1032] = [2xT_hi | 2xT_lo | -qnorm (fp32 bitcast)]
  scalar ring: cn [128, 1024] = [cT_hi | cT_lo | -cnorm replicated (fp32)]

Per-core device graph (logits = (2x).c - ||x||^2 - ||c||^2, computed as
outT[q, c] then transposed on gather), per q-chunk n:
  - three accumulating bf16 matmuls into the chunk's own PSUM bank
  - DVE bias_n[p,c] = -qnorm[p,n] + -cnorm[c]; DVE add psum_n + bias_n
  - output DMA on alternating rings, overlapping later chunks' compute
"""

import ml_dtypes
import numpy as np
from contextlib import ExitStack

import concourse.bass as bass
import concourse.bacc as bacc
import concourse.tile as tile
from concourse import mybir
from concourse.bass_utils import run_bass_kernel_spmd

Q, C, D = 4096, 256, 128
NCORES = 8
QL = Q // NCORES      # 512 queries per core
NQ = QL // 128        # 4 query chunks per core
F32 = mybir.dt.float32
BF16 = mybir.dt.bfloat16

NRM = 2 * NQ              # -qnorm: NQ fp32 values = 2*NQ bf16 cols
XCOLS = 2 * QL + NRM      # hi | lo | norms
CCOLS = 2 * C + 2 * C     # c_hi | c_lo | fp32 cnorm replica

_NC = None
LAST_RESULTS = None


def _build_nc():
    nc = bacc.Bacc("TRN2", target_bir_lowering=False)
    xq = nc.declare_dram_parameter("xq", [D, XCOLS], BF16, isOutput=False)
    cn = nc.declare_dram_parameter("cn", [D, CCOLS], BF16, isOutput=False)
    out = nc.declare_dram_parameter("out", [QL, C], F32, isOutput=True)

    with ExitStack() as ctx:
        tc = ctx.enter_context(tile.TileContext(nc))
        const = ctx.enter_context(tc.tile_pool(name="const", bufs=1))
        outp = ctx.enter_context(tc.tile_pool(name="outp", bufs=4))
        bp = ctx.enter_context(tc.tile_pool(name="bp", bufs=4))
        pm = ctx.enter_context(
            tc.tile_pool(name="pm", bufs=4, space=bass.MemorySpace.PSUM)
        )

        # PE warmup: the HAM clock gate only releases 2.4 GHz after ~3.4us of
        # sustained matmul activity. The PE would otherwise idle for ~5us
        # waiting on the input DMAs, so the real mains would all run at the
        # cold 1.2 GHz. Burn the wait on dependency-free dummy matmuls into a
        # scratch PSUM bank; the warm state survives the <3.4us gap until the
        # real mains issue.
        warm_in = const.tile([128, 256], BF16)
        nc.gpsimd.memset(warm_in[:], 1.0)
        wps = ctx.enter_context(
            tc.tile_pool(name="wps", bufs=1, space=bass.MemorySpace.PSUM)
        ).tile([128, 256], F32)
        for _ in range(18):
            nc.tensor.matmul(
                wps[:], warm_in[:, 0:128], warm_in[:], start=True, stop=True
            )

        cn_sb = const.tile([D, CCOLS], BF16)
        nc.scalar.dma_start(cn_sb[:], cn[:, :])
        xq_sb = const.tile([D, XCOLS], BF16)
        nc.sync.dma_start(xq_sb[:], xq[:, :])

        c_hi = cn_sb[:, 0:C]
        c_lo = cn_sb[:, C : 2 * C]
        ncr = cn_sb[:, 2 * C : CCOLS].bitcast(F32)                  # [128, C]
        nqn = xq_sb[:, 2 * QL : XCOLS].bitcast(F32)                 # [128, NQ]

        # bias[p,n,c] = -qnorm[p,n] + -cnorm[c]
        bias = bp.tile([128, NQ, C], F32)
        nc.vector.tensor_add(
            bias[:],
            nqn.rearrange("p (n o) -> p n o", o=1).broadcast_to([128, NQ, C]),
            ncr.rearrange("p (o c) -> p o c", o=1).broadcast_to([128, NQ, C]),
        )

        # chunk-pipelined mains + epilogue; each chunk owns a PSUM bank
        for n in range(NQ):
            x_hi = xq_sb[:, bass.ts(n, 128)]
            x_lo = xq_sb[:, QL + n * 128 : QL + (n + 1) * 128]
            ps = pm.tile([128, C], F32, tag="ps")
            nc.tensor.matmul(ps[:], x_hi, c_hi, start=True, stop=False)
            nc.tensor.matmul(ps[:], x_hi, c_lo, start=False, stop=False)
            nc.tensor.matmul(ps[:], x_lo, c_hi, start=False, stop=True)
            o = outp.tile([128, C], F32, tag="o")
            nc.vector.tensor_add(o[:], ps[:], bias[:, n, :])
            eng = nc.sync if n % 2 == 0 else nc.scalar
            eng.dma_start(out[bass.ts(n, 128), :], o[:])

    nc.compile()  # Bacc register allocation; walrus rejects unallocated regs
    return nc


def get_nc():
    global _NC
    if _NC is None:
        _NC = _build_nc()
    return _NC


def _split_bf16(a):
    """a (fp32) -> (hi, lo) bf16 with hi + lo ~= a to ~2^-18 rel."""
    hi = a.astype(ml_dtypes.bfloat16)
    lo = (a - hi.astype(np.float32)).astype(ml_dtypes.bfloat16)
    return hi, lo


def _pack_inputs(inputs, centers):
    cT = np.ascontiguousarray(centers.T)                              # [D, C]
    c_hi, c_lo = _split_bf16(cT)
    ncn = -(centers.astype(np.float64) ** 2).sum(1).astype(np.float32)
    ncr = np.ascontiguousarray(
        np.broadcast_to(ncn[None, :], (128, C)).astype(np.float32)
    )
    cn_pack = np.ascontiguousarray(
        np.concatenate([c_hi, c_lo, ncr.view(ml_dtypes.bfloat16)], axis=1)
    )
    maps = []
    for i in range(NCORES):
        xs = inputs[i * QL : (i + 1) * QL]
        x_hi, x_lo = _split_bf16(np.ascontiguousarray(2.0 * xs.T))  # [D, QL]
        nq = -(xs.astype(np.float64) ** 2).sum(1).astype(np.float32)
        nq_cols = np.ascontiguousarray(nq.reshape(NQ, 128).T)       # [128, NQ]
        xq_pack = np.ascontiguousarray(np.concatenate(
            [x_hi, x_lo, nq_cols.view(ml_dtypes.bfloat16)], axis=1
        ))
        maps.append({"xq": xq_pack, "cn": cn_pack})
    return maps


def kernel(inputs: np.ndarray, centers: np.ndarray, trace: bool = False):
    global LAST_RESULTS
    inputs = np.asarray(inputs, dtype=np.float32)
    centers = np.asarray(centers, dtype=np.float32)
    assert inputs.shape == (Q, D) and centers.shape == (C, D)

    nc_ = get_nc()
    in_maps = _pack_inputs(inputs, centers)
    res = run_bass_kernel_spmd(nc_, in_maps, list(range(NCORES)), trace=trace)
    LAST_RESULTS = res
    full = np.empty((C, Q), dtype=np.float32)
    for i in range(NCORES):
        full[:, i * QL : (i + 1) * QL] = res.results[i]["out"].T
    return full
